# revision 26
# baseline (speedup 1.0000x reference)
"""Trainium2 Bass kernel for the LP contrastive loss.

loss = mean_b( -log( pos_min_b / (pos_min_b + neg_sum_b + 1e-6) + 1e-6 ) )
  with E = exp(feats @ fs.T / TEMP), pos/neg split by label equality.

Sharding: the support set (N = Bs*TOPK = 16384) is split across the 8
cores (2048 columns each); every core keeps the full query batch
B = 2048 and computes a [2048 x 2048] slice of the similarity matrix.
Host combines the per-core partials (min of mins, sum of sums) and
applies the final -log(...)/mean in float64.

fp8 DoubleRow matmul; the PE streams ~2 moving columns/cycle in
DoubleRow (~110ns per 512-wide matmul measured), so the 64-tile sweep
floor is ~56us/core at 8 k-pairs, ~63us at 9.

Final design (MODE="s", "sorted-diagonal"): the loss is a mean over
query rows and a sum/min over support columns, so BOTH sides may be
permuted freely on the host.  Queries are sorted by label and each
core's support shard is sorted by class; every 128-query block's
positives then live in the 1-2 "diagonal" support tiles listed in
diag[mb] (computed from the actual labels at kernel build time, so
this stays correct for any input).  The one-hot mask pair
(lhs +128*onehot(labels), rhs -24*onehot(labels_s), PSUM offset
-3072 => exp shift e^-60) and the DVE row-min run ONLY on those ~19
of 64 tiles; the other ~45 tiles are plain 8-matmul tiles with a
single Act consumer:
  PE : 8 DoubleRow MMs off-diag / 9 on-diag      (~530 vs 576 MMs)
  Act: ONE Exp direct from PSUM + accum row-sum  (every tile; masked
       positives ~e^-55 vanish from the sum but stay ordered, above
       fp32 underflow e^-87 and below any negative >= e^-20)
  DVE: row-min of bf16 e_t (2x mode), diag tiles only; rows whose
       positives sit in the block's other diag tile contribute a
       negative-min >= e^-20 that can never win the final min.
The host recovers pos_min via log(minv) in float64; measured
end-to-end rel err 7.76e-5 (gate 2e-2).

TimelineSim (cost model): 56.8us/sweep (vs 61.6 for the always-masked
"eb"); HW best 68.2us because the chip P0-downclocks the PE to
~2.0GHz under sustained DoubleRow load.  Sustained throughput also
drifts +-40% with device power/thermal state, so test.py reports the
median of 3 paired measurements.

Alternative modes kept for reference: "s2" (row-sum on DVE instead of
Act accum - model-identical 56.8us, HW-indistinguishable from "s"),
"eb" (mask pair + DVE min on every tile), "a9" (v-domain min, mask
product -16384, DVE+Act both read PSUM), "e" (fp32 e_t), "eb2" (both
reduces on DVE - model-worse, tensor_reduce gets no bf16 2x), "a9p"
(paired 2-bank consumers), "d"/"base" (bf16 mask image + DVE
mask-add; "d" uses tensor_tensor_reduce which wedges real HW when
reading PSUM - do not use).

PSUM holds gamma*s (gamma=1024, inputs pre-scaled by 32 before fp8
quantization).  Host combines cores (min of mins, sum of sums) in
float64.
"""

import sys

sys.path.insert(0, "/opt/trn_rl_repo")

import numpy as np
import ml_dtypes

TEMP = 0.05
SCALE = 1.0 / TEMP  # 20.0
NCORES = 8
ALPHA = 32.0
GAMMA = ALPHA * ALPHA  # PSUM holds GAMMA * s
BIG = 16.0  # mask offset in s-units; mask value is -GAMMA*BIG = -16384

MODE = "s"  # "s" | "eb" | "e" | "a9" | "a9p" | "d" | "base"
EBIG = 3072.0  # e-family mask offset in PSUM units: exp shift of -60

_CACHE = {}


def _build(B, C, Nsh, reps=1, unroll=1, mode=MODE, diag=None):
    import contextlib

    import concourse.tile as tile
    from concourse import bacc, mybir

    dt = mybir.dt
    MT = B // 128
    KP = C // 256 + (1 if mode in ("a9", "a9p", "e", "eb", "eb2", "h", "s", "s2") else 0)  # DoubleRow k-pairs
    KT = 2 * KP
    NT = Nsh // 512
    CH = 512
    MC = B // CH

    nc = bacc.Bacc("TRN2", target_bir_lowering=False, debug=False, num_devices=NCORES)

    featsL = nc.dram_tensor(
        "featsL", [128, MC, KT, CH], dt.float8e4, kind="ExternalInput"
    ).ap()
    fsL = nc.dram_tensor(
        "fsL", [128, NT, KT, 512], dt.float8e4, kind="ExternalInput"
    ).ap()
    if mode not in ("a9", "a9p", "e", "eb", "eb2", "h", "s", "s2"):
        # host-precomputed mask image, n-major consume order:
        # masksD[p, n, m, j] = -16384 if labels[m*128+p] == labels_s[n*512+j]
        masksD = nc.dram_tensor(
            "masksD", [128, NT, MT, 512], dt.bfloat16, kind="ExternalInput"
        ).ap()
    minv_d = nc.dram_tensor("minv", [128, MT], dt.float32, kind="ExternalOutput").ap()
    sums_d = nc.dram_tensor("sums", [128, MT], dt.float32, kind="ExternalOutput").ap()

    with tile.TileContext(nc) as tc:
        with (
            tc.tile_pool(name="res", bufs=1) as res,
            tc.tile_pool(name="work", bufs=4) as work,
            tc.tile_pool(
                name="ps", bufs=(4 if mode == "a9p" else 8), space="PSUM"
            ) as psum,
        ):
            # --- resident tiles, DMA'd in the order compute consumes them ---
            lhs_t = [None] * MC
            rhs_t = [None] * NT
            if mode not in ("a9", "a9p", "e", "eb", "eb2", "h", "s", "s2"):
                masks_t = res.tile([128, NT, MT, 512], dt.bfloat16, tag="masks")

            rhs_t[0] = res.tile([128, KT, 512], dt.float8e4, name="rhs0", tag="rhs0")
            nc.sync.dma_start(rhs_t[0][:], fsL[:, 0, :, :])
            lhs_t[0] = res.tile([128, KT, CH], dt.float8e4, name="lhs0", tag="lhs0")
            nc.sync.dma_start(lhs_t[0][:], featsL[:, 0, :, :])
            if mode not in ("a9", "a9p", "e", "eb", "eb2", "h", "s", "s2"):
                nc.sync.dma_start(masks_t[:, 0, 0:4, :], masksD[:, 0, 0:4, :])
            for c in range(1, MC):
                lhs_t[c] = res.tile(
                    [128, KT, CH], dt.float8e4, name=f"lhs{c}", tag=f"lhs{c}"
                )
                nc.sync.dma_start(lhs_t[c][:], featsL[:, c, :, :])
            if mode not in ("a9", "a9p", "e", "eb", "eb2", "h", "s", "s2"):
                nc.sync.dma_start(masks_t[:, 0, 4:MT, :], masksD[:, 0, 4:MT, :])
            for n in range(1, NT):
                rhs_t[n] = res.tile(
                    [128, KT, 512], dt.float8e4, name=f"rhs{n}", tag=f"rhs{n}"
                )
                nc.sync.dma_start(rhs_t[n][:], fsL[:, n, :, :])
                if mode not in ("a9", "a9p", "e", "eb", "eb2", "h", "s", "s2"):
                    nc.sync.dma_start(masks_t[:, n, :, :], masksD[:, n, :, :])

            NCOL = NT // 2 if mode == "a9p" else NT
            mincols = res.tile([128, MT, NCOL], dt.float32, tag="mincols")
            if mode in ("s", "s2"):
                nc.vector.memset(mincols[:], 3.0e38)
            sumcols = res.tile([128, MT, NCOL], dt.float32, tag="sumcols")
            minv_t = res.tile([128, MT], dt.float32, tag="minv")
            sums_t = res.tile([128, MT], dt.float32, tag="sums")
            dummy_t = res.tile([128, 1], dt.float32, tag="dummy")

            warm = res.tile([128, 512], dt.bfloat16, tag="warm")
            nc.vector.memset(warm[:], 0.0)
            if mode == "a9p":
                wps_t = psum.tile([128, 1024], dt.float32, name="wps", tag="ps2")
                wps = wps_t[:, 0:512]
            else:
                wps_t = psum.tile([128, 512], dt.float32, name="wps", tag="ps")
                wps = wps_t[:]
            for w in range(30):
                nc.tensor.matmul(
                    wps, warm[:, 0:128], warm[:], start=(w == 0), stop=(w == 29)
                )

            def pair_body(n2, m):
                # two n-tiles' matmul groups into one flat 2-bank PSUM tile,
                # then ONE DVE min + ONE Act exp over the 1024-wide pair
                c, ci = divmod(m * 128, CH)
                ps2 = psum.tile([128, 1024], dt.float32, tag="ps2")
                for half in range(2):
                    n = 2 * n2 + half
                    for kk in range(KP):
                        nc.tensor.matmul(
                            ps2[:, 512 * half : 512 * half + 512],
                            lhs_t[c][:, 2 * kk : 2 * kk + 2, ci : ci + 128],
                            rhs_t[n][:, 2 * kk : 2 * kk + 2, :],
                            start=(kk == 0),
                            stop=(kk == KP - 1),
                            perf_mode=mybir.MatmulPerfMode.DoubleRow,
                        )
                nc.vector.tensor_reduce(
                    mincols[:, m, n2 : n2 + 1],
                    ps2[:],
                    axis=mybir.AxisListType.X,
                    op=mybir.AluOpType.min,
                )
                e_t = work.tile([128, 1024], dt.float32, tag="e2")
                nc.scalar.activation(
                    e_t[:],
                    ps2[:],
                    mybir.ActivationFunctionType.Exp,
                    scale=SCALE / GAMMA,
                    accum_out=sumcols[:, m, n2 : n2 + 1],
                )

            def tile_body(n, m):
                if mode == "a9p":
                    return pair_body(n, m)
                c, ci = divmod(m * 128, CH)
                use9 = mode in ("s", "s2") and n in diag[m]
                kp_eff = 9 if use9 else (8 if mode in ("s", "s2") else KP)
                ps = psum.tile([128, 512], dt.float32, tag="ps")
                for kk in range(kp_eff):
                    nc.tensor.matmul(
                        ps[:],
                        lhs_t[c][:, 2 * kk : 2 * kk + 2, ci : ci + 128],
                        rhs_t[n][:, 2 * kk : 2 * kk + 2, :],
                        start=(kk == 0),
                        stop=(kk == kp_eff - 1),
                        perf_mode=mybir.MatmulPerfMode.DoubleRow,
                    )
                if mode in ("s", "s2"):
                    # Act is the only per-tile PSUM consumer; DVE min runs
                    # only on this block's diagonal tiles (where all of its
                    # rows' positives provably live).  "s2" also moves the
                    # row-sum from Act's accumulator (279ns readout/tile) to
                    # the now mostly-idle DVE.
                    e_t = work.tile([128, 512], dt.bfloat16, tag="e")
                    nc.scalar.activation(
                        e_t[:],
                        ps[:],
                        mybir.ActivationFunctionType.Exp,
                        scale=SCALE / GAMMA,
                        accum_out=None if mode == "s2" else sumcols[:, m, n : n + 1],
                    )
                    if mode == "s2":
                        nc.vector.tensor_reduce(
                            sumcols[:, m, n : n + 1],
                            e_t[:],
                            axis=mybir.AxisListType.X,
                            op=mybir.AluOpType.add,
                        )
                    if use9:
                        nc.vector.tensor_reduce(
                            mincols[:, m, n : n + 1],
                            e_t[:],
                            axis=mybir.AxisListType.X,
                            op=mybir.AluOpType.min,
                        )
                elif mode == "d":
                    nc.vector.tensor_tensor_reduce(
                        dummy_t.broadcast_to((128, 512)),
                        ps[:],
                        masks_t[:, n, m, :],
                        scale=1.0,
                        scalar=3.0e38,
                        op0=mybir.AluOpType.add,
                        op1=mybir.AluOpType.min,
                        accum_out=mincols[:, m, n : n + 1],
                    )
                    e_t = work.tile([128, 512], dt.float32, tag="e")
                    nc.scalar.activation(
                        e_t[:],
                        ps[:],
                        mybir.ActivationFunctionType.Exp,
                        scale=SCALE / GAMMA,
                        accum_out=sumcols[:, m, n : n + 1],
                    )
                elif mode in ("e", "eb", "eb2", "h"):
                    # single PSUM reader: Act computes exp (positives are
                    # shifted to ~e^-55 by the one-hot matmul pair, far below
                    # any negative >= e^-20 but above fp32 underflow), then
                    # DVE takes the row-min of e_t from SBUF = the hardest
                    # positive in the exp domain.
                    edt = dt.bfloat16 if mode in ("eb", "eb2") else dt.float32
                    e_t = work.tile([128, 512], edt, tag="e")
                    nc.scalar.activation(
                        e_t[:],
                        ps[:],
                        mybir.ActivationFunctionType.Exp,
                        scale=SCALE / GAMMA,
                        accum_out=(
                            None if mode in ("h", "eb2")
                            else sumcols[:, m, n : n + 1]
                        ),
                    )
                    nc.vector.tensor_reduce(
                        mincols[:, m, n : n + 1],
                        e_t[:],
                        axis=mybir.AxisListType.X,
                        op=mybir.AluOpType.min,
                    )
                    if mode == "eb2":
                        # both reductions on DVE (bf16 2x mode) so the Act
                        # engine skips the ~279ns accumulator readout per tile
                        nc.vector.tensor_reduce(
                            sumcols[:, m, n : n + 1],
                            e_t[:],
                            axis=mybir.AxisListType.X,
                            op=mybir.AluOpType.add,
                        )
                elif mode == "a9":
                    nc.vector.tensor_reduce(
                        mincols[:, m, n : n + 1],
                        ps[:],
                        axis=mybir.AxisListType.X,
                        op=mybir.AluOpType.min,
                    )
                    e_t = work.tile([128, 512], dt.float32, tag="e")
                    nc.scalar.activation(
                        e_t[:],
                        ps[:],
                        mybir.ActivationFunctionType.Exp,
                        scale=SCALE / GAMMA,
                        accum_out=sumcols[:, m, n : n + 1],
                    )
                else:  # base
                    v_t = work.tile([128, 512], dt.float32, tag="v")
                    nc.vector.tensor_tensor(
                        v_t[:], ps[:], masks_t[:, n, m, :], mybir.AluOpType.add
                    )
                    nc.vector.tensor_reduce(
                        mincols[:, m, n : n + 1],
                        v_t[:],
                        axis=mybir.AxisListType.X,
                        op=mybir.AluOpType.min,
                    )
                    e_t = work.tile([128, 512], dt.float32, tag="e")
                    nc.scalar.activation(
                        e_t[:],
                        v_t[:],
                        mybir.ActivationFunctionType.Exp,
                        scale=SCALE / GAMMA,
                        accum_out=sumcols[:, m, n : n + 1],
                    )

            n_loop, n_flat = divmod(reps, unroll)
            rep_loop = (
                tc.For_i(
                    0,
                    n_loop,
                    1,
                    hint_engines=(mybir.EngineType.PE, mybir.EngineType.DVE),
                )
                if n_loop > 1 or (n_loop == 1 and n_flat)
                else contextlib.nullcontext()
            )
            NITER = NT // 2 if mode == "a9p" else NT
            with rep_loop:
                for _u in range(unroll if n_loop else 0):
                    for n in range(NITER):
                        for m in range(MT):
                            tile_body(n, m)

            for _f in range(n_flat):
                for n in range(NITER):
                    for m in range(MT):
                        tile_body(n, m)

            nc.vector.tensor_reduce(
                minv_t[:], mincols[:], axis=mybir.AxisListType.X, op=mybir.AluOpType.min
            )
            nc.vector.tensor_reduce(
                sums_t[:], sumcols[:], axis=mybir.AxisListType.X, op=mybir.AluOpType.add
            )
            nc.sync.dma_start(minv_d[:], minv_t[:])
            nc.sync.dma_start(sums_d[:], sums_t[:])

    nc.compile()
    return nc


def get_nc(B, C, Nsh, reps=1, unroll=16, mode=MODE, diag=None):
    key = (B, C, Nsh, reps, unroll, mode, diag)
    if key not in _CACHE:
        _CACHE[key] = _build(B, C, Nsh, reps, unroll, mode, diag)
    return _CACHE[key]


def _pack(matT, nchunks, chunk, KT):
    return np.ascontiguousarray(
        matT.reshape(KT, 128, nchunks, chunk).transpose(1, 2, 0, 3)
    )


def make_in_maps(feats, feats_s, labels, labels_s, mode=MODE):
    feats = np.asarray(feats, dtype=np.float32)
    fs = np.asarray(feats_s, dtype=np.float32).reshape(-1, feats.shape[1])
    labels = np.asarray(labels).astype(np.int64)
    labels_s = np.asarray(labels_s).astype(np.int64)

    B, C = feats.shape
    N = fs.shape[0]
    Nsh = N // NCORES
    MT = B // 128
    NT = Nsh // 512
    fp8 = ml_dtypes.float8_e4m3

    diag = None
    if mode in ("s", "s2"):
        # the loss is a mean over rows and a sum/min over support columns,
        # so both sides can be permuted freely.  Sort queries by label and
        # each core's support shard by class: every 128-query block's
        # positives then live in the few "diagonal" tiles listed in diag[mb],
        # and only those tiles need the one-hot mask pair + DVE min.
        qperm = np.argsort(labels, kind="stable")
        feats = feats[qperm]
        labels = labels[qperm]
        sperm = np.concatenate(
            [
                i * Nsh + np.argsort(labels_s[i * Nsh : (i + 1) * Nsh], kind="stable")
                for i in range(NCORES)
            ]
        )
        fs = fs[sperm]
        labels_s = labels_s[sperm]
        # per-core class -> column-range -> tile set
        cls_tiles = [set() for _ in range(256)]
        for i in range(NCORES):
            sl = labels_s[i * Nsh : (i + 1) * Nsh]
            counts = np.bincount(sl, minlength=256)
            starts = np.concatenate([[0], np.cumsum(counts)[:-1]])
            for cidx in range(256):
                if counts[cidx]:
                    lo = int(starts[cidx]) // 512
                    hi = int(starts[cidx] + counts[cidx] - 1) // 512
                    cls_tiles[cidx].update(range(lo, hi + 1))
        diag = tuple(
            tuple(
                sorted(
                    set().union(
                        *(cls_tiles[int(cl)] for cl in np.unique(labels[128 * mb : 128 * (mb + 1)]))
                    )
                )
            )
            for mb in range(MT)
        )

    bound = float(
        np.linalg.norm(feats, axis=1).max() * np.linalg.norm(fs, axis=1).max()
    )
    alpha = ALPHA / max(1.0, np.sqrt(bound))
    gamma = alpha * alpha

    featsT = feats.T * alpha  # [C, B]
    fsT_all = fs.T * alpha  # [C, N]

    if mode in ("a9", "a9p", "e", "eb", "eb2", "h", "s", "s2"):
        # mask folded into the matmul: one extra DoubleRow k-pair of
        # one-hot label rows; product is exactly -16384 (a9) / -3072 (e)
        # at positives.  Both factors are exactly representable in e4m3.
        ncls = 256
        sval = -(EBIG / 128.0) if mode in ("e", "eb", "eb2", "h", "s", "s2") else -128.0
        onehot_q = (np.arange(ncls)[:, None] == labels[None, :]).astype(
            np.float32
        ) * 128.0
        onehot_s = (np.arange(ncls)[:, None] == labels_s[None, :]).astype(
            np.float32
        ) * sval
        featsT = np.vstack([featsT, onehot_q])  # [C+256, B]
        fsT_all = np.vstack([fsT_all, onehot_s])  # [C+256, N]
        KT = (C + 256) // 128
        maskval = -EBIG if mode in ("e", "eb", "eb2", "h", "s", "s2") else -16384.0
    else:
        KT = C // 128
        # mask value must be exactly representable in bf16 so the host-side
        # un-offset (SCALE*big) matches what the DVE actually added
        maskval = float(ml_dtypes.bfloat16(-BIG * max(1.0, bound) * gamma))
    big = -maskval / gamma

    featsL = _pack(featsT.astype(fp8), B // 512, 512, KT)
    fsT_all = fsT_all.astype(fp8)

    in_maps = []
    for i in range(NCORES):
        sl = slice(i * Nsh, (i + 1) * Nsh)
        m = {"featsL": featsL, "fsL": _pack(fsT_all[:, sl], NT, 512, KT)}
        if mode not in ("a9", "a9p", "e", "eb", "eb2", "h", "s", "s2"):
            # mask image [128, NT, MT, 512]: [p, n, m, j] for query m*128+p,
            # support col n*512+j of this core's shard
            is_pos = labels[:, None] == labels_s[None, sl]  # [B, Nsh]
            mask = np.where(is_pos, np.float32(maskval), np.float32(0.0))
            m["masksD"] = np.ascontiguousarray(
                mask.reshape(MT, 128, NT, 512).transpose(1, 2, 0, 3)
            ).astype(ml_dtypes.bfloat16)
        in_maps.append(m)

    meta = {"big": big, "gamma": gamma, "mode": mode, "diag": diag}
    if mode == "d":
        # exact per-row count of positive support columns, for the host-side
        # removal of the positives' expected contribution to the full row-sum
        meta["cnt"] = (labels[:, None] == labels_s[None, :]).sum(axis=1)
        meta["N"] = N
    return in_maps, B, C, Nsh, meta


def finish_on_host(results, B, meta):
    bigv, gamma, mode = meta["big"], meta["gamma"], meta["mode"]
    MT = B // 128
    minv = np.stack([r["minv"].T.reshape(B) for r in results])
    sums = np.stack([r["sums"].T.reshape(B) for r in results])
    if mode in ("e", "eb", "eb2", "h", "s", "s2"):
        # minv is the hardest positive in the exp domain (shifted by the
        # -EBIG one-hot pair); map back to v-units via log
        vmin = np.log(minv.min(axis=0).astype(np.float64)) / (SCALE / GAMMA) / gamma
    else:
        vmin = minv.min(axis=0).astype(np.float64) / gamma
    tot_sum = sums.astype(np.float64).sum(axis=0)
    if mode == "d":
        # tot_sum includes the positives; subtract their expected share
        # (labels are independent of the features, so positives are an
        # exchangeable random subset of the row)
        frac = meta["cnt"].astype(np.float64) / float(meta["N"])
        neg_sum = tot_sum * (1.0 - frac)
    else:
        neg_sum = tot_sum
    with np.errstate(over="ignore", invalid="ignore"):
        pos_min = np.exp(SCALE * vmin + SCALE * bigv)
        loss = -np.log(pos_min / (pos_min + neg_sum + 1e-6) + 1e-6)
    return np.float32(loss.mean())


def kernel(**inputs):
    from concourse.bass_utils import run_bass_kernel_spmd

    in_maps, B, C, Nsh, meta = make_in_maps(
        inputs["feats"], inputs["feats_s"], inputs["labels"], inputs["labels_s"]
    )
    nc = get_nc(B, C, Nsh, diag=meta.get("diag"))
    res = run_bass_kernel_spmd(nc, in_maps, core_ids=list(range(NCORES)))
    return finish_on_host(res.results, B, meta)
